# revision 1
# baseline (speedup 1.0000x reference)
"""AdaptiveSamplingMixing — Trainium2 8-core SPMD kernel.

Core c = 2*b + hn handles image b (of 4) and query-half hn (150 queries).
The device kernel runs the dominant-memory stage: the output projection
(h_flat [150, 32768] @ op_w [32768, 256], K-accumulated on PE), plus the
residual add and the final affine LayerNorm, fully data-parallel (no
collectives).  Upstream stages (sampling offsets, bilinear gather, adaptive
mixing) are prepared per-shard on the host and shipped as the kernel's
h_flat input.
"""
import sys
sys.path.insert(0, "/opt/trn_rl_repo")
import numpy as np
import ml_dtypes

import concourse.bass as bass
import concourse.mybir as mybir
import concourse.tile as tile
from concourse import bacc
from concourse.bass_utils import run_bass_kernel_spmd

F32 = mybir.dt.float32
BF16 = mybir.dt.bfloat16
AL = mybir.AluOpType
AF = mybir.ActivationFunctionType

B, N, D = 4, 300, 256
G, PIN, POUT = 4, 32, 128
CG = D // G
TOTAL = CG * CG + PIN * POUT
STRIDES = (8, 16, 32, 64)
TAU = 2.0
MAP_STRIDE = 3.0
NH = N // 2  # 150 queries per core
K = G * POUT * CG  # 32768 contraction dim
KC = K // 128  # 256 K-chunks

_CACHE = {}


def _build():
    if "nc" in _CACHE:
        return _CACHE["nc"]
    nc = bacc.Bacc(None, target_bir_lowering=False, debug=True)
    hfT = nc.declare_dram_parameter("hfT", [KC, 128, NH], F32, isOutput=False)
    opw = nc.declare_dram_parameter("opw", [KC, 128, D], F32, isOutput=False)
    qf = nc.declare_dram_parameter("qf", [NH, D], F32, isOutput=False)
    lnc = nc.declare_dram_parameter("lnc", [3, 128, D], F32, isOutput=False)
    out_ext = nc.declare_dram_parameter("out", [NH, D], F32, isOutput=True)

    with tile.TileContext(nc) as tc:
        with (
            tc.tile_pool(name="w", bufs=4) as wp,
            tc.tile_pool(name="a", bufs=4) as ap_,
            tc.tile_pool(name="m", bufs=2) as mp,
            tc.tile_pool(name="ps", bufs=2, space="PSUM") as psp,
        ):
            TN = 75
            ps0 = psp.tile([TN, D], F32, tag="ps0")
            ps1 = psp.tile([TN, D], F32, tag="ps1")
            pss = [ps0, ps1]
            for ch in range(KC):
                wt = wp.tile([128, D], BF16, tag="wt")
                nc.gpsimd.dma_start(wt[:], opw[ch])
                at = ap_.tile([128, NH], BF16, tag="at")
                nc.gpsimd.dma_start(at[:], hfT[ch])
                for t in range(2):
                    nc.tensor.matmul(pss[t][:], at[:, t * TN:(t + 1) * TN], wt[:],
                                     start=(ch == 0), stop=(ch == KC - 1))
            for t in range(2):
                sl = slice(t * TN, (t + 1) * TN)
                res = mp.tile([TN, D], F32, tag="res")
                qt = mp.tile([TN, D], F32, tag="qt")
                nc.sync.dma_start(qt[:], qf[sl, :])
                nc.vector.tensor_tensor(res[:], pss[t][:], qt[:], AL.add)
                opb = mp.tile([TN, D], F32, tag="opb")
                nc.sync.dma_start(opb[:], lnc[2, :TN])
                nc.vector.tensor_tensor(res[:], res[:], opb[:], AL.add)
                s1 = mp.tile([TN, 1], F32, tag="s1")
                nc.vector.tensor_reduce(s1[:], res[:], mybir.AxisListType.X, AL.add)
                sq = mp.tile([TN, D], F32, tag="sq")
                nc.scalar.activation(sq[:], res[:], AF.Square)
                s2 = mp.tile([TN, 1], F32, tag="s2")
                nc.vector.tensor_reduce(s2[:], sq[:], mybir.AxisListType.X, AL.add)
                mu = mp.tile([TN, 1], F32, tag="mu")
                nc.any.tensor_scalar(mu[:], s1[:], 1.0 / D, None, AL.mult)
                ex2 = mp.tile([TN, 1], F32, tag="ex2")
                nc.any.tensor_scalar(ex2[:], s2[:], 1.0 / D, None, AL.mult)
                var = mp.tile([TN, 1], F32, tag="var")
                nc.vector.tensor_tensor(var[:], mu[:], mu[:], AL.mult)
                nc.vector.tensor_tensor(var[:], ex2[:], var[:], AL.subtract)
                nc.any.tensor_scalar(var[:], var[:], 1e-5, None, AL.add)
                nc.scalar.activation(var[:], var[:], AF.Sqrt)
                rr = mp.tile([TN, 1], F32, tag="rr")
                nc.vector.reciprocal(rr[:], var[:])
                nmr = mp.tile([TN, 1], F32, tag="nmr")
                nc.vector.tensor_tensor(nmr[:], mu[:], rr[:], AL.mult)
                nc.any.tensor_scalar(nmr[:], nmr[:], -1.0, None, AL.mult)
                xn = mp.tile([TN, D], F32, tag="xn")
                nc.any.tensor_scalar(xn[:], res[:], rr[:, :1], nmr[:, :1], AL.mult, AL.add)
                lg = mp.tile([TN, D], F32, tag="lg")
                nc.sync.dma_start(lg[:], lnc[0, :TN])
                lb = mp.tile([TN, D], F32, tag="lb")
                nc.sync.dma_start(lb[:], lnc[1, :TN])
                nc.vector.tensor_tensor(xn[:], xn[:], lg[:], AL.mult)
                nc.vector.tensor_tensor(xn[:], xn[:], lb[:], AL.add)
                nc.sync.dma_start(out_ext[sl, :], xn[:])
    nc.compile()
    _CACHE["nc"] = nc
    return nc


def _host_upstream(feats, query_feat, query_roi, off_w, off_b, pg_w, pg_b):
    """numpy: sampling + adaptive mixing up to h_flat [B, N, K]."""
    qf = query_feat
    offset = (qf @ off_w + off_b).reshape(B, N, G * PIN, 3)
    roi_cc = query_roi[..., :2]
    scale = 2.0 ** query_roi[..., 2:3]
    ratio = 2.0 ** np.concatenate(
        [query_roi[..., 3:4] * -0.5, query_roi[..., 3:4] * 0.5], axis=-1)
    roi_wh = scale * ratio
    sample_xy = roi_cc[:, :, None, :] + offset[..., :2] * roi_wh[:, :, None, :]
    sample_z = query_roi[..., 2:3] + offset[..., 2]
    lvl = np.arange(len(STRIDES), dtype=sample_z.dtype)
    logits = -((sample_z - MAP_STRIDE)[..., None] - lvl) ** 2 / TAU
    logits -= logits.max(-1, keepdims=True)
    e = np.exp(logits)
    lw = e / e.sum(-1, keepdims=True)
    sx = sample_xy[..., 0].reshape(B, N, G, PIN)
    sy = sample_xy[..., 1].reshape(B, N, G, PIN)
    sampled = np.zeros((B, N, G, PIN, CG), np.float32)
    for li, (feat, stride) in enumerate(zip(feats, STRIDES)):
        H, W = feat.shape[2], feat.shape[3]
        v = feat.reshape(B, G, CG, H * W)
        px = sx / stride - 0.5
        py = sy / stride - 0.5
        x0 = np.floor(px); y0 = np.floor(py)
        wx1 = px - x0; wy1 = py - y0
        wl = lw[..., li].reshape(B, N, G, PIN)
        for dx, dy, cw in ((0, 0, (1 - wx1) * (1 - wy1)), (1, 0, wx1 * (1 - wy1)),
                           (0, 1, (1 - wx1) * wy1), (1, 1, wx1 * wy1)):
            xi = (x0 + dx).astype(np.int64)
            yi = (y0 + dy).astype(np.int64)
            valid = (xi >= 0) & (xi < W) & (yi >= 0) & (yi < H)
            idx = np.clip(yi, 0, H - 1) * W + np.clip(xi, 0, W - 1)  # [B,N,G,PIN]
            g = np.take_along_axis(
                v.transpose(0, 1, 3, 2).reshape(B, G, H * W, CG)[:, None],
                idx.transpose(0, 2, 1, 3).reshape(B, G, 1, N * PIN, 1).transpose(0, 2, 1, 3, 4).reshape(B, 1, G, N * PIN, 1).transpose(0, 2, 3, 1, 4).reshape(B, G, N * PIN, 1)[:, :, None, :, :].reshape(B, G, 1, N * PIN, 1)[:, :, 0],
                axis=2,
            ) if False else None
            # straightforward gather
            vg = v.transpose(0, 1, 3, 2)  # [B,G,HW,CG]
            g = np.empty((B, G, N, PIN, CG), np.float32)
            for b in range(B):
                for gg in range(G):
                    g[b, gg] = vg[b, gg][idx[b, :, gg, :]]
            g = g.transpose(0, 2, 1, 3, 4)  # [B,N,G,PIN,CG]
            sampled += g * (cw * valid * wl)[..., None]
    params = (qf @ pg_w + pg_b).reshape(B, N, G, TOTAL)
    M = params[..., :CG * CG].reshape(B, N, G, CG, CG)
    S = params[..., CG * CG:].reshape(B, N, G, POUT, PIN)

    def ln2(x):
        mu = x.mean(axis=(-2, -1), keepdims=True)
        var = ((x - mu) ** 2).mean(axis=(-2, -1), keepdims=True)
        return (x - mu) / np.sqrt(var + 1e-5)

    h = np.einsum('bngpc,bngcd->bngpd', sampled, M)
    h = np.maximum(ln2(h), 0.0)
    h = np.einsum('bngop,bngpd->bngod', S, h)
    h = np.maximum(ln2(h), 0.0)
    return h.reshape(B, N, K).astype(np.float32)


def kernel(feat0, feat1, feat2, feat3, query_feat, query_roi,
           off_w, off_b, pg_w, pg_b, op_w, op_b, ln_g, ln_b):
    feats = [np.asarray(f, np.float32) for f in (feat0, feat1, feat2, feat3)]
    query_feat = np.asarray(query_feat, np.float32)
    query_roi = np.asarray(query_roi, np.float32)
    h_flat = _host_upstream(feats, query_feat, query_roi,
                            np.asarray(off_w, np.float32), np.asarray(off_b, np.float32),
                            np.asarray(pg_w, np.float32), np.asarray(pg_b, np.float32))
    op_w = np.asarray(op_w, np.float32)
    lncs = np.ascontiguousarray(np.broadcast_to(
        np.stack([np.asarray(ln_g, np.float32), np.asarray(ln_b, np.float32),
                  np.asarray(op_b, np.float32)])[:, None, :], (3, 128, D)))
    opw_t = np.ascontiguousarray(op_w.reshape(KC, 128, D))

    nc = _build()
    in_maps = []
    for c in range(8):
        b, hn = divmod(c, 2)
        sl = slice(hn * NH, (hn + 1) * NH)
        hfT = np.ascontiguousarray(
            h_flat[b, sl].T.reshape(KC, 128, NH))
        in_maps.append({
            "hfT": hfT,
            "opw": opw_t,
            "qf": np.ascontiguousarray(query_feat[b, sl]),
            "lnc": lncs,
        })
    res = run_bass_kernel_spmd(nc, in_maps, core_ids=list(range(8)))
    outs = res.results
    full = np.zeros((B, N, D), np.float32)
    for c in range(8):
        b, hn = divmod(c, 2)
        o = outs[c]["out"] if isinstance(outs[c], dict) else outs[c][0]
        full[b, hn * NH:(hn + 1) * NH] = np.asarray(o).reshape(NH, D)
    return full



# revision 15
# speedup vs baseline: 5.2732x; 5.2732x over previous
"""AdaptiveSamplingMixing — Trainium2 8-core SPMD kernel.

Sharding: core = bh*4 + g  (bh in {0,1} = image pair, g in {0..3} = sampling
group).  Each core processes Q=600 queries (2 images x 300) for one group:
  - bilinear sampling of its group's feature channels via dma_gather
  - adaptive mixing (params GEMM, per-query M/S matmuls, both 2D layernorms)
  - its group's slice of the output projection (K-partial)
The host computes only addressing metadata (sample indices / bilinear weights),
reshapes/casts inputs, and finishes with the 4-way partial-sum + residual +
final affine LayerNorm (a ~1 MFLOP epilogue).

rsqrt factors of both inner layernorms are folded out algebraically:
LN2(r*X) == LN2(X) for per-query scales, so mix1's r1 is dropped entirely and
mix2's r2 is applied as a per-query column scale after the projection matmul.
"""
import sys
sys.path.insert(0, "/opt/trn_rl_repo")
import numpy as np
import ml_dtypes

import concourse.bass as bass
import concourse.mybir as mybir
import concourse.tile as tile
from concourse import bacc
from concourse.bass_utils import run_bass_kernel_spmd
from concourse.masks import make_identity

F32 = mybir.dt.float32
BF16 = mybir.dt.bfloat16
I16 = mybir.dt.int16
AL = mybir.AluOpType
AF = mybir.ActivationFunctionType
AX = mybir.AxisListType
BF = ml_dtypes.bfloat16

B, N, D = 4, 300, 256
G, PIN, POUT = 4, 32, 128
CG = D // G  # 64
STRIDES = (8, 16, 32, 64)
SIZES = ((100, 160), (50, 80), (25, 40), (13, 20))
TAU, MAP_STRIDE = 2.0, 3.0

QI = 300                  # queries per image
Q = 2 * QI                # queries per core
ROWS_IMG = sum(h * w for h, w in SIZES)          # 21260
LVL_BASE = (0, 16000, 20000, 21000)
LVL_ROWS = (16000, 4000, 1000, 260)
FLATC = 2 * ROWS_IMG * CG // 128                 # 21260 (flat cols of feat view)
KCH = 64                  # proj contraction chunks of 128 (8192 total)
NPT = QI * PIN            # 9600 gather indices per (img, lvl, corner)

_CACHE = {}
import os
STAGE = int(os.environ.get("KSTAGE", "8"))


def _build():
    if "nc" in _CACHE:
        return _CACHE["nc"]
    nc = bacc.Bacc(None, target_bir_lowering=False, debug=False)

    fb = nc.declare_dram_parameter("fb", [128, FLATC], BF16, isOutput=False)
    idx_in = nc.declare_dram_parameter("idx", [16, 32 * 600], I16, isOutput=False)
    ridx_in = nc.declare_dram_parameter("ridx", [16, 40], I16, isOutput=False)
    cw_in = nc.declare_dram_parameter("cw", [128, 32 * 75], BF16, isOutput=False)
    pgw_in = nc.declare_dram_parameter("pgw", [2, 128, 8192], BF16, isOutput=False)
    pgb_in = nc.declare_dram_parameter("pgb", [1, 8192], BF16, isOutput=False)
    qft_in = nc.declare_dram_parameter("qft", [2, 128, Q], BF16, isOutput=False)
    opw_in = nc.declare_dram_parameter("opw", [KCH, 128, D], BF16, isOutput=False)
    e2_in = nc.declare_dram_parameter("e2", [64, 2], F32, isOutput=False)
    e2t_in = nc.declare_dram_parameter("e2t", [2, 64], F32, isOutput=False)
    out_ext = nc.declare_dram_parameter("out", [2, 128, Q], F32, isOutput=True)

    with tile.TileContext(nc) as tc:
        with (
            tc.tile_pool(name="dram", bufs=1, space="DRAM") as dp,
            tc.tile_pool(name="const", bufs=1) as cp,
        ):
            fs32 = dp.tile([2 * ROWS_IMG, CG], F32, tag="fs32")
            pdram = dp.tile([Q, 8192], BF16, tag="pdram")
            hdram = dp.tile([KCH, Q, 128], BF16, tag="hdram")

            ident = cp.tile([128, 128], F32, tag="ident")
            make_identity(nc, ident[:, :])
            e2 = cp.tile([64, 2], F32, tag="e2")
            nc.sync.dma_start(e2[:, :], e2_in[:, :])
            e2t = cp.tile([2, 64], F32, tag="e2t")
            nc.sync.dma_start(e2t[:, :], e2t_in[:, :])
            ones_f = cp.tile([1, 128], F32, tag="ones_f")
            nc.vector.memset(ones_f[:, :], 1.0)
            ones_b = cp.tile([1, 128], BF16, tag="ones_b")
            nc.vector.memset(ones_b[:, :], 1.0)
            onesc_f = cp.tile([128, 1], F32, tag="onesc_f")
            nc.vector.memset(onesc_f[:, :], 1.0)
            ridx = cp.tile([128, 40], I16, tag="ridx")
            for r in range(8):
                nc.sync.dma_start(ridx[r * 16:(r + 1) * 16, :], ridx_in[:, :])
            cw = cp.tile([128, 32 * 75], BF16, tag="cw")
            nc.sync.dma_start(cw[:, :], cw_in[:, :])
            r2e_all = cp.tile([128, Q], F32, tag="r2e")

            # ---- Phase A: expand bf16 features to f32 gather source ----
            fs_flat = fs32[:, :].rearrange("r c -> (r c)").rearrange(
                "(p i) -> p i", p=128)
            with tc.tile_pool(name="pa", bufs=2) as pa:
                CH = FLATC // 4
                for ch in range(4):
                    t16 = pa.tile([128, CH], BF16, tag="t16")
                    nc.sync.dma_start(t16[:, :], fb[:, ch * CH:(ch + 1) * CH])
                    t32 = pa.tile([128, CH], F32, tag="t32")
                    nc.vector.tensor_copy(t32[:, :], t16[:, :])
                    nc.sync.dma_start(fs_flat[:, ch * CH:(ch + 1) * CH], t32[:, :])

            # ---- Phase B: params GEMM -> pdram [Q, 8192] (q-major, bf16) ----
            if STAGE >= 2:
              with (
                tc.tile_pool(name="pb", bufs=2) as pb,
                tc.tile_pool(name="pbw", bufs=1) as pbw,
                tc.tile_pool(name="psb", bufs=4, space="PSUM") as psb,
              ):
                pgw_sb = []
                for k in range(2):
                    w = pbw.tile([128, 8192], BF16, tag=f"pgw{k}")
                    nc.sync.dma_start(w[:, :], pgw_in[k])
                    pgw_sb.append(w)
                pgb_sb = pbw.tile([1, 8192], BF16, tag="pgb")
                nc.sync.dma_start(pgb_sb[:, :], pgb_in[:, :])
                qft_sb = []
                for k in range(2):
                    w = pbw.tile([128, Q], BF16, tag=f"qft{k}")
                    nc.sync.dma_start(w[:, :], qft_in[k])
                    qft_sb.append(w)
                for qb in range(5):
                    qs = slice(qb * 120, (qb + 1) * 120)
                    qsb = pb.tile([120, 8192], BF16, tag="qsb")
                    for cb in range(16):
                        cs = slice(cb * 512, (cb + 1) * 512)
                        ps = psb.tile([120, 512], F32, tag="ps")
                        nc.tensor.matmul(ps[:, :], qft_sb[0][:, qs],
                                         pgw_sb[0][:, cs], start=True, stop=False)
                        nc.tensor.matmul(ps[:, :], qft_sb[1][:, qs],
                                         pgw_sb[1][:, cs], start=False, stop=False)
                        nc.tensor.matmul(ps[:, :], ones_b[0:1, :120],
                                         pgb_sb[0:1, cs], start=False, stop=True)
                        nc.any.tensor_copy(qsb[:, cs], ps[:, :])
                    nc.sync.dma_start(pdram[qs, :], qsb[:, :])

            # ---- Phase C: per image: gather+combine, mix1, LN, mix2, LN ----
            if STAGE >= 3:
              with (
                tc.tile_pool(name="pidx", bufs=2) as pidx,
                tc.tile_pool(name="pg", bufs=1) as pg,
                tc.tile_pool(name="pacc", bufs=1) as pacc,
                tc.tile_pool(name="pms", bufs=2) as pms,
                tc.tile_pool(name="pst", bufs=3) as pstp,
                tc.tile_pool(name="ph", bufs=1) as ph,
                tc.tile_pool(name="psq", bufs=1) as psqp,
                tc.tile_pool(name="psmall", bufs=2) as psm,
                tc.tile_pool(name="psc", bufs=1, space="PSUM") as psc,
                tc.tile_pool(name="psh2", bufs=2, space="PSUM") as psh2,
                tc.tile_pool(name="psc2", bufs=2, space="PSUM") as psc2,
                tc.tile_pool(name="psms", bufs=1, space="PSUM") as psms,
              ):
                for img in range(2):
                    qoff = img * QI
                    acc = pacc.tile([128, 75, CG], F32, tag="acc")
                    for li in range(4):
                        base = img * ROWS_IMG + LVL_BASE[li]
                        rows = LVL_ROWS[li]
                        idx_sb = pidx.tile([128, 2400], I16, tag="idx")
                        for r in range(8):
                            nc.sync.dma_start(
                                idx_sb[r * 16:(r + 1) * 16, :],
                                idx_in[:, img * 9600 + li * 2400:
                                       img * 9600 + (li + 1) * 2400])
                        for c4 in range(4):
                            ci = img * 16 + li * 4 + c4      # global call id
                            v = pg.tile([128, 75, CG], F32, tag="v")
                            # dma_gather is limited to 1024 idxs per call
                            for cc in range(10):
                                nn = 1024 if cc < 9 else 384
                                nc.gpsimd.dma_gather(
                                    v[:, cc * 8:cc * 8 + nn // 128, :],
                                    fs32[base:base + rows, :],
                                    idx_sb[:, c4 * 600 + cc * 64:
                                           c4 * 600 + cc * 64 + nn // 16],
                                    nn, nn, CG)
                            wexp = cw[:, ci * 75:(ci + 1) * 75].unsqueeze(
                                2).to_broadcast([128, 75, CG])
                            nc.vector.tensor_tensor(v[:, :, :], v[:, :, :],
                                                    wexp, AL.mult)
                            if li == 0 and c4 == 0:
                                nc.vector.tensor_copy(acc[:, :, :], v[:, :, :])
                            else:
                                nc.vector.tensor_tensor(
                                    acc[:, :, :], acc[:, :, :], v[:, :, :],
                                    AL.add)

                    if STAGE < 4:
                        continue
                    # transposes + M loads + mix1 (5-qc batches)
                    # queries are 4-stacked (q4,pin) on partitions by the
                    # gather layout; matmul OUT bases may be 0/32 but matmul
                    # OPERANDS must start at partition 0, so h1r and S live
                    # as four per-q4 32-partition tiles.
                    h1A = ph.tile([CG, 75, CG], BF16, tag="h1A")
                    h1B = ph.tile([CG, 75, CG], BF16, tag="h1B")
                    for qcb in range(15):
                        mi = pms.tile([CG, 20, CG], BF16, tag="mi")
                        nc.sync.dma_start(
                            mi[:, :, :],
                            pdram[qoff + qcb * 20:qoff + (qcb + 1) * 20,
                                  0:4096].rearrange("i (c d) -> c i d", c=CG))
                        h1psA = psc.tile([CG, 5, CG], F32, tag="h1psA")
                        h1psB = psc.tile([CG, 5, CG], F32, tag="h1psB")
                        for j in range(5):
                            qc = qcb * 5 + j
                            pst = psc2.tile([CG, 128], F32, tag="pst")
                            nc.tensor.transpose(pst[:, :], acc[:, qc, :],
                                                ident[:, :])
                            sT = pstp.tile([CG, 128], BF16, tag="sT")
                            nc.any.tensor_copy(sT[:, :], pst[:, :])
                            for q4 in range(4):
                                hp = h1psA if q4 < 2 else h1psB
                                pb = (q4 % 2) * PIN
                                nc.tensor.matmul(
                                    hp[pb:pb + PIN, j, :],
                                    sT[:, q4 * PIN:(q4 + 1) * PIN],
                                    mi[:, j * 4 + q4, :],
                                    start=True, stop=True)
                        nc.any.tensor_copy(h1A[:, qcb * 5:(qcb + 1) * 5, :],
                                           h1psA[:, :, :])
                        nc.any.tensor_copy(h1B[:, qcb * 5:(qcb + 1) * 5, :],
                                           h1psB[:, :, :])

                    # LN#1 per half (over p,d per query; rsqrt folded out)
                    h1rs = []
                    for hi, h1h in enumerate((h1A, h1B)):
                        h1d = psm.tile([CG, 75], F32, tag="h1d")
                        nc.vector.tensor_reduce(h1d[:, :].unsqueeze(2),
                                                h1h[:, :, :], AX.X, AL.add)
                        sqd = psm.tile([CG, 75], F32, tag="sqd")
                        for kk in range(3):
                            sl = slice(kk * 25, (kk + 1) * 25)
                            sq = psqp.tile([128, 25 * CG], F32, tag="sq")
                            nc.scalar.activation(
                                sq[:CG, :],
                                h1h[:, sl, :].rearrange("p a b -> p (a b)"),
                                AF.Square)
                            nc.vector.tensor_reduce(
                                sqd[:, sl].unsqueeze(2),
                                sq[:CG, :].rearrange("p (a b) -> p a b",
                                                     b=CG),
                                AX.X, AL.add)
                        s1p = psms.tile([128, QI], F32, tag="pmm")
                        nc.tensor.matmul(s1p[:2, :75], e2[:, :], h1d[:, :],
                                         start=True, stop=True)
                        mu1 = psm.tile([2, 75], F32, tag="mu1")
                        nc.any.tensor_scalar(mu1[:, :], s1p[:2, :75],
                                             1.0 / 2048.0, None, AL.mult)
                        m1e = psms.tile([128, QI], F32, tag="pmm")
                        nc.tensor.matmul(m1e[:CG, :75], e2t[:, :], mu1[:, :],
                                         start=True, stop=True)
                        mu1e = psm.tile([CG, 75], F32, tag="mu1e")
                        nc.any.tensor_copy(mu1e[:, :], m1e[:CG, :75])
                        for hq in range(2):
                            q4 = hi * 2 + hq
                            pb = hq * PIN
                            h1r = ph.tile([PIN, 75, CG], BF16,
                                          tag=f"h1rq{q4}")
                            nc.vector.tensor_tensor(
                                h1r[:, :, :], h1h[pb:pb + PIN, :, :],
                                mu1e[pb:pb + PIN, :].unsqueeze(
                                    2).to_broadcast([PIN, 75, CG]),
                                AL.subtract)
                            nc.any.tensor_scalar(
                                h1r[:, :, :].rearrange("p a b -> p (a b)"),
                                h1r[:, :, :].rearrange("p a b -> p (a b)"),
                                0.0, None, AL.max)
                            h1rs.append(h1r)

                    if STAGE < 5:
                        continue
                    # mix2: h2[q] = S_q @ h1r_q  -> h2sb [128(o-perm), 300*64]
                    h2sb = ph.tile([128, QI, CG], BF16, tag="h2sb")
                    for qcb in range(15):
                        blk = pdram[qoff + qcb * 20:qoff + (qcb + 1) * 20,
                                    4096:8192].rearrange(
                            "(i q) (p o) -> q p i o", i=5, p=PIN)
                        sis = []
                        for q4 in range(4):
                            si = pms.tile([PIN, 5, 128], BF16,
                                          tag=f"siq{q4}")
                            nc.sync.dma_start(si[:, :, :], blk[q4])
                            sis.append(si)
                        for jj in range(4):
                            h2ps = psh2.tile([128, 5, CG], F32, tag="h2ps")
                            for j in range(5):
                                i20 = jj * 5 + j
                                i5 = i20 // 4
                                qc = qcb * 5 + i5
                                q4 = i20 % 4
                                nc.tensor.matmul(
                                    h2ps[:, j, :],
                                    sis[q4][:, i5, :],
                                    h1rs[q4][:, qc, :],
                                    start=True, stop=True)
                            nc.any.tensor_copy(
                                h2sb[:, qcb * 20 + jj * 5:
                                     qcb * 20 + (jj + 1) * 5, :],
                                h2ps[:, :, :])

                    if STAGE < 6:
                        continue
                    # LN#2 stats (over o,d per query)
                    h2d = psm.tile([128, QI], F32, tag="h2d")
                    nc.vector.tensor_reduce(h2d[:, :].unsqueeze(2),
                                            h2sb[:, :, :], AX.X, AL.add)
                    sqd2 = psm.tile([128, QI], F32, tag="sqd2")
                    for kk in range(12):
                        sl = slice(kk * 25, (kk + 1) * 25)
                        sq2 = psqp.tile([128, 25 * CG], F32, tag="sq")
                        nc.scalar.activation(
                            sq2[:, :],
                            h2sb[:, sl, :].rearrange("p a b -> p (a b)"),
                            AF.Square)
                        nc.vector.tensor_reduce(
                            sqd2[:, sl].unsqueeze(2),
                            sq2[:, :].rearrange("p (a b) -> p a b", b=CG),
                            AX.X, AL.add)
                    s1q = psms.tile([128, QI], F32, tag="pmm")
                    nc.tensor.matmul(s1q[:1, :], onesc_f[:, :], h2d[:, :],
                                     start=True, stop=True)
                    s2q = psms.tile([128, QI], F32, tag="pmm")
                    nc.tensor.matmul(s2q[:1, :], onesc_f[:, :], sqd2[:, :],
                                     start=True, stop=True)
                    mu2 = psm.tile([1, QI], F32, tag="mu2")
                    nc.any.tensor_scalar(mu2[:, :], s1q[:1, :], 1.0 / 8192.0,
                                         None, AL.mult)
                    ex2 = psm.tile([1, QI], F32, tag="ex2")
                    nc.any.tensor_scalar(ex2[:, :], s2q[:1, :], 1.0 / 8192.0,
                                         None, AL.mult)
                    var2 = psm.tile([1, QI], F32, tag="var2")
                    nc.vector.tensor_tensor(var2[:, :], mu2[:, :], mu2[:, :],
                                            AL.mult)
                    nc.vector.tensor_tensor(var2[:, :], ex2[:, :], var2[:, :],
                                            AL.subtract)
                    r2 = psm.tile([1, QI], F32, tag="r2")
                    nc.any.tensor_scalar(var2[:, :], var2[:, :], 1e-5,
                                         None, AL.add)
                    nc.scalar.activation(r2[:, :], var2[:, :], AF.Sqrt)
                    nc.vector.reciprocal(r2[:, :], r2[:, :])
                    m2e = psms.tile([128, QI], F32, tag="pmm")
                    nc.tensor.matmul(m2e[:, :], ones_f[:, :], mu2[:, :],
                                     start=True, stop=True)
                    mu2e = psm.tile([128, QI], F32, tag="mu2e")
                    nc.any.tensor_copy(mu2e[:, :], m2e[:, :])
                    r2ep = psms.tile([128, QI], F32, tag="pmm")
                    nc.tensor.matmul(r2ep[:, :], ones_f[:, :], r2[:, :],
                                     start=True, stop=True)
                    nc.any.tensor_copy(r2e_all[:, qoff:qoff + QI], r2ep[:, :])

                    if STAGE < 7:
                        continue
                    # h2r = relu(h2 - mu2) in place, then store chunk-major
                    nc.vector.tensor_tensor(
                        h2sb[:, :, :], h2sb[:, :, :],
                        mu2e[:, :].unsqueeze(2).to_broadcast([128, QI, CG]),
                        AL.subtract)
                    nc.any.tensor_scalar(
                        h2sb[:, :, :].rearrange("p a b -> p (a b)"),
                        h2sb[:, :, :].rearrange("p a b -> p (a b)"),
                        0.0, None, AL.max)
                    nc.sync.dma_start(hdram[:, qoff:qoff + QI, 0:64],
                                      h2sb[0:64, :, :])
                    nc.sync.dma_start(hdram[:, qoff:qoff + QI, 64:128],
                                      h2sb[64:128, :, :])

            # ---- Phase D: output projection with K accumulation ----
            if STAGE >= 8:
              with (
                tc.tile_pool(name="pd", bufs=3) as pd,
                tc.tile_pool(name="pdw", bufs=4) as pdw,
                tc.tile_pool(name="pso", bufs=1, space="PSUM") as pso,
                tc.tile_pool(name="pout", bufs=2) as pout,
              ):
                prps = [[None, None], [None, None]]
                for dh in range(2):
                    for qh in range(2):
                        prtile = pso.tile([128, QI], F32, tag=f"pr{dh}{qh}")
                        prps[dh][qh] = prtile
                for t in range(KCH):
                    rt = pd.tile([128, 1, 640], BF16, tag="rt")
                    nc.gpsimd.dma_gather(
                        rt[:, :, :], hdram[t], ridx[:, :],
                        640, Q, 128, transpose=True)
                    ow = pdw.tile([128, D], BF16, tag="ow")
                    nc.sync.dma_start(ow[:, :], opw_in[t])
                    for dh in range(2):
                        for qh in range(2):
                            nc.tensor.matmul(
                                prps[dh][qh][:, :],
                                ow[:, dh * 128:(dh + 1) * 128],
                                rt[:, 0, qh * QI:(qh + 1) * QI],
                                start=(t == 0), stop=(t == KCH - 1))
                for dh in range(2):
                    for qh in range(2):
                        osb = pout.tile([128, QI], F32, tag="osb")
                        nc.vector.tensor_tensor(
                            osb[:, :], prps[dh][qh][:, :],
                            r2e_all[:, qh * QI:(qh + 1) * QI], AL.mult)
                        nc.sync.dma_start(
                            out_ext[dh, :, qh * QI:(qh + 1) * QI], osb[:, :])
    nc.compile()
    _CACHE["nc"] = nc
    return nc


def _host_prep(feats, query_feat, query_roi, off_w, off_b, pg_w, pg_b, op_w):
    """Vectorized numpy: addressing metadata + per-core input tensors."""
    qf = query_feat.astype(np.float32)
    offset = (qf @ off_w + off_b).reshape(B, N, G * PIN, 3)
    roi_cc = query_roi[..., :2]
    scale = 2.0 ** query_roi[..., 2:3]
    ratio = 2.0 ** np.concatenate(
        [query_roi[..., 3:4] * -0.5, query_roi[..., 3:4] * 0.5], axis=-1)
    roi_wh = scale * ratio
    sample_xy = roi_cc[:, :, None, :] + offset[..., :2] * roi_wh[:, :, None, :]
    sample_z = query_roi[..., 2:3] + offset[..., 2]
    lvl = np.arange(4, dtype=np.float32)
    logits = -((sample_z - MAP_STRIDE)[..., None] - lvl) ** 2 / TAU
    logits -= logits.max(-1, keepdims=True)
    e = np.exp(logits)
    lw = (e / e.sum(-1, keepdims=True)).astype(np.float32)  # [B,N,G*PIN,4]
    sx = sample_xy[..., 0]                                  # [B,N,G*PIN]
    sy = sample_xy[..., 1]

    # per (lvl, corner) indices and weights, [B, N, G*PIN]
    idx_all = np.zeros((4, 2, 2, B, N, G * PIN), np.int16)
    w_all = np.zeros((4, 2, 2, B, N, G * PIN), np.float32)
    for li, ((H, W), stride) in enumerate(zip(SIZES, STRIDES)):
        px = sx / stride - 0.5
        py = sy / stride - 0.5
        x0 = np.floor(px)
        y0 = np.floor(py)
        wx1 = px - x0
        wy1 = py - y0
        for dy in range(2):
            for dx in range(2):
                xi = (x0 + dx).astype(np.int64)
                yi = (y0 + dy).astype(np.int64)
                valid = (xi >= 0) & (xi < W) & (yi >= 0) & (yi < H)
                wc = (wx1 if dx else 1.0 - wx1) * (wy1 if dy else 1.0 - wy1)
                idx_all[li, dy, dx] = np.where(valid, yi * W + xi, 0
                                               ).astype(np.int16)
                w_all[li, dy, dx] = wc * lw[..., li] * valid

    # per-core tensors
    in_maps = []
    ew = np.zeros((64, 2), np.float32)
    for p in range(64):
        ew[p, p // 32] = 1.0
    ridx_flat = np.full(640, -1, np.int16)
    ridx_flat[:Q] = np.arange(Q, dtype=np.int16)
    ridx_wr = np.ascontiguousarray(ridx_flat.reshape(40, 16).T)

    sinv = np.empty(128, np.int64)   # output-partition permutation for mix2
    for m in range(128):
        sinv[m] = 2 * m if m < 64 else 2 * (m - 64) + 1

    scol = np.empty(4096, np.int64)  # S-part column order (p_in, o_perm)
    for p_in in range(PIN):
        for m in range(128):
            scol[p_in * 128 + m] = 4096 + sinv[m] * PIN + p_in

    for bh in range(2):
        imgs = (2 * bh, 2 * bh + 1)
        for g in range(G):
            # features: [2*21260, 64] channel-last, bf16, flat [128, FLATC]
            fparts = []
            for b in imgs:
                for li, (H, W) in enumerate(SIZES):
                    f = feats[li][b, g * CG:(g + 1) * CG]  # [64, H, W]
                    fparts.append(f.reshape(CG, H * W).T)
                # level order must match LVL_BASE: contiguous concat
            fcat = np.concatenate(fparts, axis=0).astype(BF)  # [42520, 64]
            fb = np.ascontiguousarray(fcat.reshape(128, FLATC))

            # gather idx / weights: call ci = img*16 + li*4 + (dy*2+dx)
            idx_cols = np.empty((16, 32 * 600), np.int16)
            cw_cols = np.empty((128, 32 * 75), BF)
            for ii, b in enumerate(imgs):
                for li in range(4):
                    for c4 in range(4):
                        dy, dx = c4 // 2, c4 % 2
                        ci = ii * 16 + li * 4 + c4
                        ia = idx_all[li, dy, dx, b, :,
                                     g * PIN:(g + 1) * PIN]   # [300, 32]
                        idx_cols[:, ci * 600:(ci + 1) * 600] = \
                            ia.reshape(NPT).reshape(600, 16).T
                        wa = w_all[li, dy, dx, b, :,
                                   g * PIN:(g + 1) * PIN]     # [300, 32]
                        cw_cols[:, ci * 75:(ci + 1) * 75] = \
                            wa.reshape(75, 4, PIN).transpose(1, 2, 0).reshape(
                                128, 75).astype(BF)

            cols = np.concatenate([np.arange(4096), scol])
            pgw_c = pg_w[:, g * 8192:(g + 1) * 8192][:, cols].astype(BF)
            pgw_c = np.ascontiguousarray(pgw_c.reshape(2, 128, 8192))
            pgb_c = pg_b[g * 8192:(g + 1) * 8192][cols].astype(BF)[None, :]
            qft = np.ascontiguousarray(
                qf[list(imgs)].reshape(Q, D).T.astype(BF).reshape(2, 128, Q))
            opw_c = np.ascontiguousarray(
                op_w[g * 8192:(g + 1) * 8192, :].astype(BF).reshape(
                    KCH, 128, D))
            in_maps.append({
                "fb": fb, "idx": np.ascontiguousarray(idx_cols),
                "ridx": ridx_wr, "cw": np.ascontiguousarray(cw_cols),
                "pgw": pgw_c, "pgb": np.ascontiguousarray(pgb_c),
                "qft": qft, "opw": opw_c,
                "e2": ew, "e2t": np.ascontiguousarray(ew.T),
            })
    return in_maps


def kernel(feat0, feat1, feat2, feat3, query_feat, query_roi,
           off_w, off_b, pg_w, pg_b, op_w, op_b, ln_g, ln_b):
    feats = [np.asarray(f, np.float32) for f in (feat0, feat1, feat2, feat3)]
    query_feat = np.asarray(query_feat, np.float32)
    query_roi = np.asarray(query_roi, np.float32)
    in_maps = _host_prep(feats, query_feat, query_roi,
                         np.asarray(off_w, np.float32),
                         np.asarray(off_b, np.float32),
                         np.asarray(pg_w, np.float32),
                         np.asarray(pg_b, np.float32),
                         np.asarray(op_w, np.float32))
    nc = _build()
    res = run_bass_kernel_spmd(nc, in_maps, core_ids=list(range(8)))
    outs = res.results

    op_b = np.asarray(op_b, np.float32)
    ln_g = np.asarray(ln_g, np.float32)
    ln_b = np.asarray(ln_b, np.float32)
    full = np.zeros((B, N, D), np.float32)
    for b in range(B):
        bh, img = b // 2, b % 2
        acc = np.zeros((D, N), np.float32)
        for g in range(G):
            o = outs[bh * 4 + g]
            o = o["out"] if isinstance(o, dict) else o[0]
            o = np.asarray(o, np.float32).reshape(2, 128, Q)
            acc += o[:, :, img * QI:(img + 1) * QI].reshape(D, QI)
        h = acc.T + query_feat[b] + op_b
        mu = h.mean(-1, keepdims=True)
        var = ((h - mu) ** 2).mean(-1, keepdims=True)
        full[b] = (h - mu) / np.sqrt(var + 1e-5) * ln_g + ln_b
    return full
